# revision 29
# baseline (speedup 1.0000x reference)
"""Deformable conv v1 Bass/Tile kernel for TRN2 (one sample per core).

Host prep (layout-only): quad table ptab[yy*65+xs] = 4 zero-padded corner
pixel vectors (TL,TR,BL,BR) x 256ch bf16; offsets in both [18,HW] and
transposed [HW,18] layouts; weights pre-transposed bf16.

Device pipeline per core:
  coords: bilinear corner weights W4d (pixel%128 on partitions, bf16,
          duplicated in pairs so the broadcast mult hits DVE 2x_1P mode) and
          gather idx tiles (L2 16-wrap layout -> PE fold/replicate -> int16)
  main:   per 1024-pixel chunk, per tap: dma_gather of 2KB corner quads
          (SWDGE queues round-robin) -> pair-packed tensor_tensor mult +
          halving adds into cols[pix, k, ch]
          per 512-pixel chunk: HWDGE xbar dma_start_transpose cols ->
          colsT[ck%128, (grp, k*2+cb), pix], 18-step matmul accumulation
          -> PSUM -> out
"""
import numpy as np
import ml_dtypes

import concourse.bass as bass
import concourse.tile as tile
from concourse import bacc, mybir

F32 = mybir.dt.float32
BF16 = mybir.dt.bfloat16
I16 = mybir.dt.int16

P = 128
H = W = 64
HW = H * W          # 4096
C = 256
O = 256
KK = 9              # 3x3 taps
NTAB = (H + 1) * (W + 1)  # 4225 quad entries
GCH = 1024          # gather num_idxs granularity
SCH = 512           # cols/GEMM chunk
BIG = 12582912.0    # 1.5*2^23 round-trick constant (ulp=1 for |v| <= 2^22)
NQ = 3              # SWDGE queues

Alu = mybir.AluOpType


def _bases():
    """Base sampling grids. k = ky*3+kx; y_base = ky-1+row, x_base = kx-1+col."""
    n = np.arange(HW)
    ky = (np.arange(KK) // 3).astype(np.float32) - 1.0
    kx = (np.arange(KK) % 3).astype(np.float32) - 1.0
    yb = ky[None, :] + (n // W).astype(np.float32)[:, None]  # [HW, 9]
    xb = kx[None, :] + (n % W).astype(np.float32)[:, None]
    return yb, xb


def host_constants():
    yb, xb = _bases()
    # P-layout [128, 32, 9]: pixel n = 128*b + p
    ybp = yb.reshape(32, P, KK).transpose(1, 0, 2).copy()
    xbp = xb.reshape(32, P, KK).transpose(1, 0, 2).copy()
    # L2 layout [2, 128, 9, 16]: pixel n = 2048*cc + 16*Pp + f
    ybl = yb.reshape(2, P, 16, KK).transpose(0, 1, 3, 2).copy()
    xbl = xb.reshape(2, P, 16, KK).transpose(0, 1, 3, 2).copy()
    i128 = np.eye(P, dtype=np.float32)
    # replication matrix [16, 128]
    rep = (np.arange(P)[None, :] % 16 == np.arange(16)[:, None]).astype(np.float32)
    return dict(ybp=ybp, xbp=xbp, ybl=ybl, xbl=xbl, rep=rep, i128=i128)


def host_weight(weight: np.ndarray) -> np.ndarray:
    """weight [O, C, 3, 3] f32 -> wt [128, 18, 256] bf16; B = k*2 + cb."""
    w = weight.reshape(O, C, KK)                        # k = ky*3+kx
    wt = np.empty((P, 2 * KK, O), dtype=np.float32)
    for k in range(KK):
        for cb in range(2):
            # wt[p, k*2+cb, o] = w[o, cb*128+p, k]
            wt[:, k * 2 + cb, :] = w[:, cb * P:(cb + 1) * P, k].T
    return wt.astype(ml_dtypes.bfloat16)


def host_ptab(img: np.ndarray) -> np.ndarray:
    """img [C, H, W] f32 -> quad table [NTAB, 4*C] bf16.

    Entry (ey, ex) holds corners for y0 = ey-1, x0 = ex-1:
      slot0 TL = img[ey-1, ex-1], slot1 TR = img[ey-1, ex],
      slot2 BL = img[ey,   ex-1], slot3 BR = img[ey,   ex], OOB -> 0.
    """
    cl = np.ascontiguousarray(img.transpose(1, 2, 0)).astype(ml_dtypes.bfloat16)
    pt = np.zeros((H + 1, W + 1, 4, C), dtype=ml_dtypes.bfloat16)
    pt[1:, 1:, 0] = cl
    pt[1:, :W, 1] = cl
    pt[:H, 1:, 2] = cl
    pt[:H, :W, 3] = cl
    return pt.reshape(NTAB, 4 * C)


def host_inputs(input, offset, weight):
    """Full inputs -> per-core in_maps for run_bass_kernel_spmd."""
    input = np.ascontiguousarray(np.asarray(input, dtype=np.float32))
    offset = np.ascontiguousarray(np.asarray(offset, dtype=np.float32))
    weight = np.ascontiguousarray(np.asarray(weight, dtype=np.float32))
    wt = host_weight(weight)
    in_maps = []
    for b in range(8):
        off = offset[b].reshape(18, HW)
        # offl[pp, cc, c, f] = off[c, cc*2048 + pp*16 + f]  (L2 16-wrap)
        offl = np.ascontiguousarray(
            off.reshape(18, 2, P, 16).transpose(2, 1, 0, 3))
        # offp[p, b32, c] = off[c, b32*128 + p]  (pixel-partition)
        offp = np.ascontiguousarray(
            off.reshape(18, 32, P).transpose(2, 1, 0))
        in_maps.append({
            "ptab": host_ptab(input[b]),
            "offl": offl,
            "offp": offp,
            "wt": wt,
        })
    return in_maps


def build(num_swdge_queues=NQ):
    nc = bacc.Bacc("TRN2", target_bir_lowering=False, debug=False,
                   num_devices=8, num_swdge_queues=num_swdge_queues,
                   dynamic_dma_scratch_size=16384)
    consts = host_constants()

    ptab = nc.dram_tensor("ptab", [NTAB, 4 * C], BF16, kind="ExternalInput").ap()
    offl = nc.dram_tensor("offl", [P, 2, 18, 16], F32, kind="ExternalInput").ap()
    offp = nc.dram_tensor("offp", [P, 32, 18], F32, kind="ExternalInput").ap()
    wt_d = nc.dram_tensor("wt", [P, 18, O], BF16, kind="ExternalInput").ap()
    y = nc.dram_tensor("y", [O, HW], F32, kind="ExternalOutput").ap()

    c_i128 = nc.inline_tensor(consts["i128"], "c_i128").ap()
    c_rep = nc.inline_tensor(consts["rep"], "c_rep").ap()
    c_ybp = nc.inline_tensor(consts["ybp"], "c_ybp").ap()
    c_xbp = nc.inline_tensor(consts["xbp"], "c_xbp").ap()
    c_ybl = nc.inline_tensor(consts["ybl"], "c_ybl").ap()
    c_xbl = nc.inline_tensor(consts["xbl"], "c_xbl").ap()

    with tile.TileContext(nc) as tc:
        _body(nc, tc, ptab, offl, offp, wt_d, y,
              c_i128, c_rep, c_ybp, c_xbp, c_ybl, c_xbl)
    nc.compile()
    return nc


def _body(nc, tc, ptab, offl, offp, wt_d, y,
          c_i128, c_rep, c_ybp, c_xbp, c_ybl, c_xbl):
    import contextlib
    ctx = contextlib.ExitStack()

    # ---------------- constants + offsets into SBUF ----------------
    cpool = ctx.enter_context(tc.tile_pool(name="consts", bufs=1))
    mpool = ctx.enter_context(tc.tile_pool(name="meta", bufs=1))
    # offsets first: they gate the idx/weight math that gates the gathers.
    # host pre-laid-out for contiguous DMA (strided loads cost ~15us each)
    offL_t = mpool.tile([P, 2, 18, 16], F32, tag="offL")
    nc.sync.dma_start(offL_t[:], offl)
    offT = mpool.tile([P, 32, 18], F32, tag="offT")
    nc.scalar.dma_start(offT[:], offp)
    ybl_sb = cpool.tile([P, 2, KK, 16], F32, tag="ybl")
    nc.sync.dma_start(ybl_sb[:], c_ybl.transpose([1, 0, 2, 3]))
    xbl_sb = cpool.tile([P, 2, KK, 16], F32, tag="xbl")
    nc.sync.dma_start(xbl_sb[:], c_xbl.transpose([1, 0, 2, 3]))
    i128_sb = cpool.tile([P, P], F32, tag="i128")
    nc.sync.dma_start(i128_sb[:], c_i128)
    i128b_sb = cpool.tile([P, P], BF16, tag="i128b")
    nc.vector.tensor_copy(i128b_sb[:], i128_sb[:])
    rep_sb = cpool.tile([16, P], F32, tag="rep")
    nc.sync.dma_start(rep_sb[:], c_rep)
    wt_sb = cpool.tile([P, 18, O], BF16, tag="wt")
    nc.scalar.dma_start(wt_sb[:], wt_d)

    pp_small = ctx.enter_context(tc.tile_pool(name="ps_small", bufs=1, space="PSUM"))

    # ---------------- L2 layout -> gather idx tiles ----------------
    # idx_t[(cc, k)]: [128, 128] int16 covering 2048 idx; separate tiles so
    # each tap's gather fires as soon as its own idx lands (tile-granular deps)
    idx_t = {}
    for cc in range(2):
        for k in range(KK):
            it = mpool.tile([P, P], I16, tag=f"idx{cc}_{k}")
            idx_t[(cc, k)] = it
    lpool = ctx.enter_context(tc.tile_pool(name="l2", bufs=1))

    def emit_l2(cc):
        offL = offL_t[:, cc]

        def lfloor(v, nm):
            """floor(v) into a fresh per-axis tile; scratch tiles shared."""
            vr = lpool.tile([P, KK, 16], F32, tag="lvr")
            nc.vector.tensor_scalar(vr[:], v[:], BIG, -BIG, Alu.add, Alu.add)
            d = lpool.tile([P, KK, 16], F32, tag="ld")
            nc.vector.tensor_tensor(d[:], v[:], vr[:], Alu.subtract)
            nc.vector.tensor_scalar(d[:], d[:], 0.0, None, Alu.is_lt)
            v0 = lpool.tile([P, KK, 16], F32, tag=nm + "v0")
            nc.vector.tensor_tensor(v0[:], vr[:], d[:], Alu.subtract)
            return v0

        yv = lpool.tile([P, KK, 16], F32, tag="lyv")
        nc.vector.tensor_tensor(yv[:], offL[:, 0:18:2, :], ybl_sb[:, cc],
                                Alu.add)
        y0 = lfloor(yv, "ly")
        xv = lpool.tile([P, KK, 16], F32, tag="lxv")
        nc.vector.tensor_tensor(xv[:], offL[:, 1:18:2, :], xbl_sb[:, cc],
                                Alu.add)
        x0 = lfloor(xv, "lx")
        nc.vector.tensor_scalar(x0[:], x0[:], 1.0, 0.0, Alu.add, Alu.max)
        nc.vector.tensor_scalar(x0[:], x0[:], 64.0, None, Alu.min)
        nc.vector.tensor_scalar(y0[:], y0[:], 1.0, 0.0, Alu.add, Alu.max)
        nc.vector.tensor_scalar(y0[:], y0[:], 64.0, None, Alu.min)
        eT = lpool.tile([P, KK, 16], F32, tag="leT")
        nc.vector.scalar_tensor_tensor(eT[:], y0[:], 65.0, x0[:],
                                       Alu.mult, Alu.add)
        for k in range(KK):
            psA = pp_small.tile([P, P], F32, tag="psA")
            nc.tensor.matmul(psA[0:16, :], eT[:, k, :], i128_sb[:],
                             start=True, stop=True)
            e16 = lpool.tile([16, P], F32, tag="e16")
            nc.scalar.copy(e16[:], psA[0:16, :])
            psB = pp_small.tile([P, P], F32, tag="psB")
            nc.tensor.matmul(psB[:], rep_sb[:], e16[:],
                             start=True, stop=True)
            nc.vector.tensor_copy(idx_t[(cc, k)][:], psB[:])

    emit_l2(0)

    # ---------------- bilinear corner weights W4 (bf16) ----------------
    # NOTE: built via build_wmath() which is EMITTED after the first gather
    # so the scheduler orders the idx casts ahead of this DVE work (the
    # gather's sem threshold follows DVE stream order).
    wpool = ctx.enter_context(tc.tile_pool(name="wmath", bufs=1))
    ybp_sb = wpool.tile([P, 32, KK], F32, tag="ybp")
    nc.sync.dma_start(ybp_sb[:], c_ybp)
    xbp_sb = wpool.tile([P, 32, KK], F32, tag="xbp")
    nc.sync.dma_start(xbp_sb[:], c_xbp)

    def floor_block(v, pool, nm):
        """returns (v0 floor, frac) as new tiles, 4+1 ops"""
        vr = pool.tile([P, 32, KK], F32, tag=nm + "vr")
        nc.vector.tensor_scalar(vr[:], v[:], BIG, -BIG, Alu.add, Alu.add)
        d = pool.tile([P, 32, KK], F32, tag=nm + "d")
        nc.vector.tensor_tensor(d[:], v[:], vr[:], Alu.subtract)
        ng = pool.tile([P, 32, KK], F32, tag=nm + "ng")
        nc.vector.tensor_scalar(ng[:], d[:], 0.0, None, Alu.is_lt)
        v0 = pool.tile([P, 32, KK], F32, tag=nm + "v0")
        nc.vector.tensor_tensor(v0[:], vr[:], ng[:], Alu.subtract)
        fr = pool.tile([P, 32, KK], F32, tag=nm + "fr")
        nc.vector.tensor_tensor(fr[:], v[:], v0[:], Alu.subtract)
        return v0, fr

    def axis_weights(base_sb, chan0, nm):
        """weighted+masked pair (w_low, w_high) [128, 32, 9] for one axis."""
        v = wpool.tile([P, 32, KK], F32, tag=nm + "v")
        nc.vector.tensor_tensor(v[:], offT[:, :, chan0:18:2], base_sb[:],
                                Alu.add)
        v0, fr = floor_block(v, wpool, nm)
        wlo = wpool.tile([P, 32, KK], F32, tag=nm + "wlo")
        nc.vector.tensor_scalar(wlo[:], fr[:], -1.0, 1.0, Alu.mult, Alu.add)
        # valid low: 0 <= v0 <= 63 ; valid high: -1 <= v0 <= 62
        va = wpool.tile([P, 32, KK], F32, tag=nm + "va")
        nc.vector.tensor_scalar(va[:], v0[:], 0.0, None, Alu.is_ge)
        vb = wpool.tile([P, 32, KK], F32, tag=nm + "vb")
        nc.vector.tensor_scalar(vb[:], v0[:], 63.0, None, Alu.is_le)
        vlo = wpool.tile([P, 32, KK], F32, tag=nm + "vlo")
        nc.vector.tensor_tensor(vlo[:], va[:], vb[:], Alu.mult)
        nc.vector.tensor_scalar(va[:], v0[:], -1.0, None, Alu.is_ge)
        nc.vector.tensor_scalar(vb[:], v0[:], 62.0, None, Alu.is_le)
        vhi = wpool.tile([P, 32, KK], F32, tag=nm + "vhi")
        nc.vector.tensor_tensor(vhi[:], va[:], vb[:], Alu.mult)
        wl = wpool.tile([P, 32, KK], F32, tag=nm + "wl")
        nc.vector.tensor_tensor(wl[:], wlo[:], vlo[:], Alu.mult)
        wh = wpool.tile([P, 32, KK], F32, tag=nm + "wh")
        nc.vector.tensor_tensor(wh[:], fr[:], vhi[:], Alu.mult)
        return wl, wh

    # corners jj: 0=TL 1=TR 2=BL 3=BR, bf16, duplicated along a trailing
    # pair dim so the main mult's weight AP has inner step 1 (DVE 2x_1P)
    W4d = mpool.tile([P, 32, 4, KK, 2], BF16, tag="W4d")

    def build_wmath():
        wyl, wyh = axis_weights(ybp_sb, 0, "y")
        wxl, wxh = axis_weights(xbp_sb, 1, "x")
        for pair in range(2):
            nc.vector.tensor_tensor(W4d[:, :, 0, :, pair], wyl[:], wxl[:],
                                    Alu.mult)
            nc.vector.tensor_tensor(W4d[:, :, 1, :, pair], wyl[:], wxh[:],
                                    Alu.mult)
            nc.vector.tensor_tensor(W4d[:, :, 2, :, pair], wyh[:], wxl[:],
                                    Alu.mult)
            nc.vector.tensor_tensor(W4d[:, :, 3, :, pair], wyh[:], wxh[:],
                                    Alu.mult)

    # ---------------- main loop ----------------
    gpool = ctx.enter_context(tc.tile_pool(name="gather", bufs=3))
    tpool = ctx.enter_context(tc.tile_pool(name="tmp", bufs=1))
    ckpool = ctx.enter_context(tc.tile_pool(name="colk", bufs=3))
    ctpool = ctx.enter_context(tc.tile_pool(name="colsT", bufs=3))
    opool = ctx.enter_context(tc.tile_pool(name="outp", bufs=2))
    pp_mm = ctx.enter_context(tc.tile_pool(name="ps_mm", bufs=2, space="PSUM"))

    y_v = y.rearrange("(oh p) (s n) -> oh p s n", oh=2, n=SCH)
    pp_tr = ctx.enter_context(tc.tile_pool(name="ps_tr", bufs=2, space="PSUM"))

    # per-sub-chunk GEMM state: sch -> (ct4, ps0, ps1)
    state = {}

    def emit_tr_round(sch, k, csl):
        """Streamed per-tap transpose+GEMM: PE transpose (no gpsimd)
        colk[pix, grp, c] -> ct4[c%128, B=k*2+cb, grp, pix], then the two
        B-step GEMM accumulations for this tap."""
        if k == 0:
            ct4 = ctpool.tile([P, 18, 4, P], BF16, tag="ct4")
            ps0 = pp_mm.tile([P, SCH], F32, tag="ps0")
            ps1 = pp_mm.tile([P, SCH], F32, tag="ps1")
            state[sch] = (ct4, ps0, ps1)
        ct4, ps0, ps1 = state[sch]
        pst = pp_tr.tile([P, 2, 4, P], BF16, tag="pstr")
        for cb in range(2):
            for grp in range(4):
                nc.tensor.matmul(
                    pst[:, cb, grp, :],
                    csl[:, grp, cb * P:(cb + 1) * P],
                    i128b_sb[:], start=True, stop=True,
                    is_transpose=True)
        nc.scalar.copy(ct4[:, k * 2:k * 2 + 2, :, :], pst[:])
        for cb in range(2):
            B = k * 2 + cb
            for oh, psx in ((0, ps0), (1, ps1)):
                nc.tensor.matmul(psx[:],
                                 wt_sb[:, B, oh * P:(oh + 1) * P],
                                 ct4[:, B, :, :],
                                 start=(B == 0), stop=(B == 17))

    def finish_gemm(sch):
        _, ps0, ps1 = state.pop(sch)
        for oh, psx in ((0, ps0), (1, ps1)):
            outt = opool.tile([P, SCH], F32, tag="outt")
            nc.scalar.copy(outt[:], psx[:])
            nc.sync.dma_start(y_v[oh, :, sch, :], outt[:])

    for g in range(HW // GCH):           # 4 gather chunks of 1024 pixels
        cc = g // 2
        half = g % 2
        for k in range(KK):
            gt = gpool.tile([P, 8, 4 * C], BF16, tag="gt")
            nc.gpsimd.dma_gather(
                gt[:], ptab,
                idx_t[(cc, k)][:, half * 64:(half + 1) * 64],
                num_idxs=GCH, num_idxs_reg=GCH,
                elem_size=4 * C, elem_step=4 * C,
                queue_num=(0 if k % 2 == 0 else 2))
            if g == 0 and k == 0:
                # emitted here (not earlier) so idx casts outrank this DVE
                # work in the scheduler's per-engine stream order
                build_wmath()
            # bilinear: tmp = gt * W4d (pair-packed bcast over ch), corner adds
            gt4 = gt[:].rearrange("p c8 (j ch two) -> p (c8 j) ch two",
                                  j=4, two=2)
            w4b = W4d[:, g * 8:(g + 1) * 8, :, k:k + 1, :].rearrange(
                "p c8 j k two -> p (c8 j) k two").broadcast_to(
                [P, 32, C // 2, 2])
            tmp = tpool.tile([P, 8, 4, C], BF16, tag="btmp")
            tmp_v = tmp[:].rearrange("p c8 j (ch two) -> p (c8 j) ch two",
                                     two=2)
            nc.vector.tensor_tensor(tmp_v, gt4, w4b, Alu.mult)
            t2 = tpool.tile([P, 8, 2, C], BF16, tag="bt2")
            nc.vector.tensor_tensor(t2[:], tmp[:, :, 0:2, :],
                                    tmp[:, :, 2:4, :], Alu.add)
            colk = ckpool.tile([P, 8, C], BF16, tag="colk")
            nc.vector.tensor_tensor(colk[:], t2[:, :, 0, :],
                                    t2[:, :, 1, :], Alu.add)
            for s2 in range(2):
                emit_tr_round(g * 2 + s2, k,
                              colk[:, 4 * s2:4 * (s2 + 1), :])
        if g == 0:
            emit_l2(1)
        finish_gemm(g * 2)
        finish_gemm(g * 2 + 1)

    ctx.close()


# ---------------- harness entry point ----------------

_CACHED_NC = None


def _get_nc():
    global _CACHED_NC
    if _CACHED_NC is None:
        _CACHED_NC = build()
    return _CACHED_NC


def kernel(input, offset, weight):
    """Deformable conv v1 on 8 TRN2 cores, one sample per core.

    input  [8, 256, 64, 64] f32
    offset [8, 18, 64, 64]  f32
    weight [256, 256, 3, 3] f32
    -> [8, 256, 64, 64] f32
    """
    from concourse.bass_utils import run_bass_kernel_spmd
    nc = _get_nc()
    in_maps = host_inputs(input, offset, weight)
    res = run_bass_kernel_spmd(nc, in_maps, core_ids=list(range(8)))
    out = np.stack([res.results[b]["y"].reshape(O, H, W) for b in range(8)])
    return out



# revision 30
# speedup vs baseline: 1.0266x; 1.0266x over previous
"""Deformable conv v1 Bass/Tile kernel for TRN2 (one sample per core).

Host prep (layout-only): quad table ptab[yy*65+xs] = 4 zero-padded corner
pixel vectors (TL,TR,BL,BR) x 256ch bf16; offsets in both [18,HW] and
transposed [HW,18] layouts; weights pre-transposed bf16.

Device pipeline per core:
  coords: bilinear corner weights W4d (pixel%128 on partitions, bf16,
          duplicated in pairs so the broadcast mult hits DVE 2x_1P mode) and
          gather idx tiles (L2 16-wrap layout -> PE fold/replicate -> int16)
  main:   per 1024-pixel chunk, per tap: dma_gather of 2KB corner quads
          (SWDGE queues round-robin) -> pair-packed tensor_tensor mult +
          halving adds into cols[pix, k, ch]
          per 512-pixel chunk: HWDGE xbar dma_start_transpose cols ->
          colsT[ck%128, (grp, k*2+cb), pix], 18-step matmul accumulation
          -> PSUM -> out
"""
import numpy as np
import ml_dtypes

import concourse.bass as bass
import concourse.tile as tile
from concourse import bacc, mybir

F32 = mybir.dt.float32
BF16 = mybir.dt.bfloat16
I16 = mybir.dt.int16

P = 128
H = W = 64
HW = H * W          # 4096
C = 256
O = 256
KK = 9              # 3x3 taps
NTAB = (H + 1) * (W + 1)  # 4225 quad entries
GCH = 1024          # gather num_idxs granularity
SCH = 512           # cols/GEMM chunk
BIG = 12582912.0    # 1.5*2^23 round-trick constant (ulp=1 for |v| <= 2^22)
NQ = 3              # SWDGE queues

Alu = mybir.AluOpType


def _bases():
    """Base sampling grids. k = ky*3+kx; y_base = ky-1+row, x_base = kx-1+col."""
    n = np.arange(HW)
    ky = (np.arange(KK) // 3).astype(np.float32) - 1.0
    kx = (np.arange(KK) % 3).astype(np.float32) - 1.0
    yb = ky[None, :] + (n // W).astype(np.float32)[:, None]  # [HW, 9]
    xb = kx[None, :] + (n % W).astype(np.float32)[:, None]
    return yb, xb


def host_constants():
    yb, xb = _bases()
    # P-layout [128, 32, 9]: pixel n = 128*b + p
    ybp = yb.reshape(32, P, KK).transpose(1, 0, 2).copy()
    xbp = xb.reshape(32, P, KK).transpose(1, 0, 2).copy()
    # L2 layout [2, 128, 9, 16]: pixel n = 2048*cc + 16*Pp + f
    ybl = yb.reshape(2, P, 16, KK).transpose(0, 1, 3, 2).copy()
    xbl = xb.reshape(2, P, 16, KK).transpose(0, 1, 3, 2).copy()
    i128 = np.eye(P, dtype=np.float32)
    # replication matrix [16, 128]
    rep = (np.arange(P)[None, :] % 16 == np.arange(16)[:, None]).astype(np.float32)
    return dict(ybp=ybp, xbp=xbp, ybl=ybl, xbl=xbl, rep=rep, i128=i128)


def host_weight(weight: np.ndarray) -> np.ndarray:
    """weight [O, C, 3, 3] f32 -> wt [128, 18, 256] bf16; B = k*2 + cb."""
    w = weight.reshape(O, C, KK)                        # k = ky*3+kx
    wt = np.empty((P, 2 * KK, O), dtype=np.float32)
    for k in range(KK):
        for cb in range(2):
            # wt[p, k*2+cb, o] = w[o, cb*128+p, k]
            wt[:, k * 2 + cb, :] = w[:, cb * P:(cb + 1) * P, k].T
    return wt.astype(ml_dtypes.bfloat16)


def host_ptab(img: np.ndarray) -> np.ndarray:
    """img [C, H, W] f32 -> quad table [NTAB, 4*C] bf16.

    Entry (ey, ex) holds corners for y0 = ey-1, x0 = ex-1:
      slot0 TL = img[ey-1, ex-1], slot1 TR = img[ey-1, ex],
      slot2 BL = img[ey,   ex-1], slot3 BR = img[ey,   ex], OOB -> 0.
    """
    cl = np.ascontiguousarray(img.transpose(1, 2, 0)).astype(ml_dtypes.bfloat16)
    pt = np.zeros((H + 1, W + 1, 4, C), dtype=ml_dtypes.bfloat16)
    pt[1:, 1:, 0] = cl
    pt[1:, :W, 1] = cl
    pt[:H, 1:, 2] = cl
    pt[:H, :W, 3] = cl
    return pt.reshape(NTAB, 4 * C)


def host_inputs(input, offset, weight):
    """Full inputs -> per-core in_maps for run_bass_kernel_spmd."""
    input = np.ascontiguousarray(np.asarray(input, dtype=np.float32))
    offset = np.ascontiguousarray(np.asarray(offset, dtype=np.float32))
    weight = np.ascontiguousarray(np.asarray(weight, dtype=np.float32))
    wt = host_weight(weight)
    in_maps = []
    for b in range(8):
        off = offset[b].reshape(18, HW)
        # offl[pp, cc, c, f] = off[c, cc*2048 + pp*16 + f]  (L2 16-wrap)
        offl = np.ascontiguousarray(
            off.reshape(18, 2, P, 16).transpose(2, 1, 0, 3))
        # offp[p, b32, c] = off[c, b32*128 + p]  (pixel-partition)
        offp = np.ascontiguousarray(
            off.reshape(18, 32, P).transpose(2, 1, 0))
        in_maps.append({
            "ptab": host_ptab(input[b]),
            "offl": offl,
            "offp": offp,
            "wt": wt,
        })
    return in_maps


def build(num_swdge_queues=NQ):
    nc = bacc.Bacc("TRN2", target_bir_lowering=False, debug=False,
                   num_devices=8, num_swdge_queues=num_swdge_queues,
                   dynamic_dma_scratch_size=16384)
    consts = host_constants()

    ptab = nc.dram_tensor("ptab", [NTAB, 4 * C], BF16, kind="ExternalInput").ap()
    offl = nc.dram_tensor("offl", [P, 2, 18, 16], F32, kind="ExternalInput").ap()
    offp = nc.dram_tensor("offp", [P, 32, 18], F32, kind="ExternalInput").ap()
    wt_d = nc.dram_tensor("wt", [P, 18, O], BF16, kind="ExternalInput").ap()
    y = nc.dram_tensor("y", [O, HW], F32, kind="ExternalOutput").ap()

    c_i128 = nc.inline_tensor(consts["i128"], "c_i128").ap()
    c_rep = nc.inline_tensor(consts["rep"], "c_rep").ap()
    c_ybp = nc.inline_tensor(consts["ybp"], "c_ybp").ap()
    c_xbp = nc.inline_tensor(consts["xbp"], "c_xbp").ap()
    c_ybl = nc.inline_tensor(consts["ybl"], "c_ybl").ap()
    c_xbl = nc.inline_tensor(consts["xbl"], "c_xbl").ap()

    with tile.TileContext(nc) as tc:
        _body(nc, tc, ptab, offl, offp, wt_d, y,
              c_i128, c_rep, c_ybp, c_xbp, c_ybl, c_xbl)
    nc.compile()
    return nc


def _body(nc, tc, ptab, offl, offp, wt_d, y,
          c_i128, c_rep, c_ybp, c_xbp, c_ybl, c_xbl):
    import contextlib
    ctx = contextlib.ExitStack()

    # ---------------- constants + offsets into SBUF ----------------
    cpool = ctx.enter_context(tc.tile_pool(name="consts", bufs=1))
    mpool = ctx.enter_context(tc.tile_pool(name="meta", bufs=1))
    # offsets first: they gate the idx/weight math that gates the gathers.
    # host pre-laid-out for contiguous DMA (strided loads cost ~15us each)
    offL_t = mpool.tile([P, 2, 18, 16], F32, tag="offL")
    nc.sync.dma_start(offL_t[:], offl)
    offT = mpool.tile([P, 32, 18], F32, tag="offT")
    nc.scalar.dma_start(offT[:], offp)
    ybl_sb = cpool.tile([P, 2, KK, 16], F32, tag="ybl")
    nc.sync.dma_start(ybl_sb[:], c_ybl.transpose([1, 0, 2, 3]))
    xbl_sb = cpool.tile([P, 2, KK, 16], F32, tag="xbl")
    nc.sync.dma_start(xbl_sb[:], c_xbl.transpose([1, 0, 2, 3]))
    i128_sb = cpool.tile([P, P], F32, tag="i128")
    nc.sync.dma_start(i128_sb[:], c_i128)
    i128b_sb = cpool.tile([P, P], BF16, tag="i128b")
    nc.vector.tensor_copy(i128b_sb[:], i128_sb[:])
    rep_sb = cpool.tile([16, P], F32, tag="rep")
    nc.sync.dma_start(rep_sb[:], c_rep)
    wt_sb = cpool.tile([P, 18, O], BF16, tag="wt")
    nc.scalar.dma_start(wt_sb[:], wt_d)

    pp_small = ctx.enter_context(tc.tile_pool(name="ps_small", bufs=1, space="PSUM"))

    # ---------------- L2 layout -> gather idx tiles ----------------
    # idx_t[(cc, k)]: [128, 128] int16 covering 2048 idx; separate tiles so
    # each tap's gather fires as soon as its own idx lands (tile-granular deps)
    idx_t = {}
    for cc in range(2):
        for k in range(KK):
            it = mpool.tile([P, P], I16, tag=f"idx{cc}_{k}")
            idx_t[(cc, k)] = it
    lpool = ctx.enter_context(tc.tile_pool(name="l2", bufs=1))

    def emit_l2(cc):
        offL = offL_t[:, cc]

        def lfloor(v, nm):
            """floor(v) into a fresh per-axis tile; scratch tiles shared."""
            vr = lpool.tile([P, KK, 16], F32, tag="lvr")
            nc.vector.tensor_scalar(vr[:], v[:], BIG, -BIG, Alu.add, Alu.add)
            d = lpool.tile([P, KK, 16], F32, tag="ld")
            nc.vector.tensor_tensor(d[:], v[:], vr[:], Alu.subtract)
            nc.vector.tensor_scalar(d[:], d[:], 0.0, None, Alu.is_lt)
            v0 = lpool.tile([P, KK, 16], F32, tag=nm + "v0")
            nc.vector.tensor_tensor(v0[:], vr[:], d[:], Alu.subtract)
            return v0

        yv = lpool.tile([P, KK, 16], F32, tag="lyv")
        nc.vector.tensor_tensor(yv[:], offL[:, 0:18:2, :], ybl_sb[:, cc],
                                Alu.add)
        y0 = lfloor(yv, "ly")
        xv = lpool.tile([P, KK, 16], F32, tag="lxv")
        nc.vector.tensor_tensor(xv[:], offL[:, 1:18:2, :], xbl_sb[:, cc],
                                Alu.add)
        x0 = lfloor(xv, "lx")
        nc.vector.tensor_scalar(x0[:], x0[:], 1.0, 0.0, Alu.add, Alu.max)
        nc.vector.tensor_scalar(x0[:], x0[:], 64.0, None, Alu.min)
        nc.vector.tensor_scalar(y0[:], y0[:], 1.0, 0.0, Alu.add, Alu.max)
        nc.vector.tensor_scalar(y0[:], y0[:], 64.0, None, Alu.min)
        eT = lpool.tile([P, KK, 16], F32, tag="leT")
        nc.vector.scalar_tensor_tensor(eT[:], y0[:], 65.0, x0[:],
                                       Alu.mult, Alu.add)
        for k in range(KK):
            psA = pp_small.tile([P, P], F32, tag="psA")
            nc.tensor.matmul(psA[0:16, :], eT[:, k, :], i128_sb[:],
                             start=True, stop=True)
            e16 = lpool.tile([16, P], F32, tag="e16")
            nc.scalar.copy(e16[:], psA[0:16, :])
            psB = pp_small.tile([P, P], F32, tag="psB")
            nc.tensor.matmul(psB[:], rep_sb[:], e16[:],
                             start=True, stop=True)
            nc.vector.tensor_copy(idx_t[(cc, k)][:], psB[:])

    with tc.high_priority():
        emit_l2(0)

    # ---------------- bilinear corner weights W4 (bf16) ----------------
    # NOTE: built via build_wmath() which is EMITTED after the first gather
    # so the scheduler orders the idx casts ahead of this DVE work (the
    # gather's sem threshold follows DVE stream order).
    wpool = ctx.enter_context(tc.tile_pool(name="wmath", bufs=1))
    ybp_sb = wpool.tile([P, 32, KK], F32, tag="ybp")
    nc.sync.dma_start(ybp_sb[:], c_ybp)
    xbp_sb = wpool.tile([P, 32, KK], F32, tag="xbp")
    nc.sync.dma_start(xbp_sb[:], c_xbp)

    def floor_block(v, pool, nm):
        """returns (v0 floor, frac) as new tiles, 4+1 ops"""
        vr = pool.tile([P, 32, KK], F32, tag=nm + "vr")
        nc.vector.tensor_scalar(vr[:], v[:], BIG, -BIG, Alu.add, Alu.add)
        d = pool.tile([P, 32, KK], F32, tag=nm + "d")
        nc.vector.tensor_tensor(d[:], v[:], vr[:], Alu.subtract)
        ng = pool.tile([P, 32, KK], F32, tag=nm + "ng")
        nc.vector.tensor_scalar(ng[:], d[:], 0.0, None, Alu.is_lt)
        v0 = pool.tile([P, 32, KK], F32, tag=nm + "v0")
        nc.vector.tensor_tensor(v0[:], vr[:], ng[:], Alu.subtract)
        fr = pool.tile([P, 32, KK], F32, tag=nm + "fr")
        nc.vector.tensor_tensor(fr[:], v[:], v0[:], Alu.subtract)
        return v0, fr

    def axis_weights(base_sb, chan0, nm):
        """weighted+masked pair (w_low, w_high) [128, 32, 9] for one axis."""
        v = wpool.tile([P, 32, KK], F32, tag=nm + "v")
        nc.vector.tensor_tensor(v[:], offT[:, :, chan0:18:2], base_sb[:],
                                Alu.add)
        v0, fr = floor_block(v, wpool, nm)
        wlo = wpool.tile([P, 32, KK], F32, tag=nm + "wlo")
        nc.vector.tensor_scalar(wlo[:], fr[:], -1.0, 1.0, Alu.mult, Alu.add)
        # valid low: 0 <= v0 <= 63 ; valid high: -1 <= v0 <= 62
        va = wpool.tile([P, 32, KK], F32, tag=nm + "va")
        nc.vector.tensor_scalar(va[:], v0[:], 0.0, None, Alu.is_ge)
        vb = wpool.tile([P, 32, KK], F32, tag=nm + "vb")
        nc.vector.tensor_scalar(vb[:], v0[:], 63.0, None, Alu.is_le)
        vlo = wpool.tile([P, 32, KK], F32, tag=nm + "vlo")
        nc.vector.tensor_tensor(vlo[:], va[:], vb[:], Alu.mult)
        nc.vector.tensor_scalar(va[:], v0[:], -1.0, None, Alu.is_ge)
        nc.vector.tensor_scalar(vb[:], v0[:], 62.0, None, Alu.is_le)
        vhi = wpool.tile([P, 32, KK], F32, tag=nm + "vhi")
        nc.vector.tensor_tensor(vhi[:], va[:], vb[:], Alu.mult)
        wl = wpool.tile([P, 32, KK], F32, tag=nm + "wl")
        nc.vector.tensor_tensor(wl[:], wlo[:], vlo[:], Alu.mult)
        wh = wpool.tile([P, 32, KK], F32, tag=nm + "wh")
        nc.vector.tensor_tensor(wh[:], fr[:], vhi[:], Alu.mult)
        return wl, wh

    # corners jj: 0=TL 1=TR 2=BL 3=BR, bf16, duplicated along a trailing
    # pair dim so the main mult's weight AP has inner step 1 (DVE 2x_1P)
    W4d = mpool.tile([P, 32, 4, KK, 2], BF16, tag="W4d")

    def build_wmath():
        wyl, wyh = axis_weights(ybp_sb, 0, "y")
        wxl, wxh = axis_weights(xbp_sb, 1, "x")
        for pair in range(2):
            nc.vector.tensor_tensor(W4d[:, :, 0, :, pair], wyl[:], wxl[:],
                                    Alu.mult)
            nc.vector.tensor_tensor(W4d[:, :, 1, :, pair], wyl[:], wxh[:],
                                    Alu.mult)
            nc.vector.tensor_tensor(W4d[:, :, 2, :, pair], wyh[:], wxl[:],
                                    Alu.mult)
            nc.vector.tensor_tensor(W4d[:, :, 3, :, pair], wyh[:], wxh[:],
                                    Alu.mult)

    # ---------------- main loop ----------------
    gpool = ctx.enter_context(tc.tile_pool(name="gather", bufs=3))
    tpool = ctx.enter_context(tc.tile_pool(name="tmp", bufs=1))
    ckpool = ctx.enter_context(tc.tile_pool(name="colk", bufs=3))
    ctpool = ctx.enter_context(tc.tile_pool(name="colsT", bufs=3))
    opool = ctx.enter_context(tc.tile_pool(name="outp", bufs=2))
    pp_mm = ctx.enter_context(tc.tile_pool(name="ps_mm", bufs=2, space="PSUM"))

    y_v = y.rearrange("(oh p) (s n) -> oh p s n", oh=2, n=SCH)
    pp_tr = ctx.enter_context(tc.tile_pool(name="ps_tr", bufs=2, space="PSUM"))

    # per-sub-chunk GEMM state: sch -> (ct4, ps0, ps1)
    state = {}

    def emit_tr_round(sch, k, csl):
        """Streamed per-tap transpose+GEMM: PE transpose (no gpsimd)
        colk[pix, grp, c] -> ct4[c%128, B=k*2+cb, grp, pix], then the two
        B-step GEMM accumulations for this tap."""
        if k == 0:
            ct4 = ctpool.tile([P, 18, 4, P], BF16, tag="ct4")
            ps0 = pp_mm.tile([P, SCH], F32, tag="ps0")
            ps1 = pp_mm.tile([P, SCH], F32, tag="ps1")
            state[sch] = (ct4, ps0, ps1)
        ct4, ps0, ps1 = state[sch]
        pst = pp_tr.tile([P, 2, 4, P], BF16, tag="pstr")
        for cb in range(2):
            for grp in range(4):
                nc.tensor.matmul(
                    pst[:, cb, grp, :],
                    csl[:, grp, cb * P:(cb + 1) * P],
                    i128b_sb[:], start=True, stop=True,
                    is_transpose=True)
        nc.scalar.copy(ct4[:, k * 2:k * 2 + 2, :, :], pst[:])
        for cb in range(2):
            B = k * 2 + cb
            for oh, psx in ((0, ps0), (1, ps1)):
                nc.tensor.matmul(psx[:],
                                 wt_sb[:, B, oh * P:(oh + 1) * P],
                                 ct4[:, B, :, :],
                                 start=(B == 0), stop=(B == 17))

    def finish_gemm(sch):
        _, ps0, ps1 = state.pop(sch)
        for oh, psx in ((0, ps0), (1, ps1)):
            outt = opool.tile([P, SCH], F32, tag="outt")
            nc.scalar.copy(outt[:], psx[:])
            nc.sync.dma_start(y_v[oh, :, sch, :], outt[:])

    for g in range(HW // GCH):           # 4 gather chunks of 1024 pixels
        cc = g // 2
        half = g % 2
        for k in range(KK):
            gt = gpool.tile([P, 8, 4 * C], BF16, tag="gt")
            nc.gpsimd.dma_gather(
                gt[:], ptab,
                idx_t[(cc, k)][:, half * 64:(half + 1) * 64],
                num_idxs=GCH, num_idxs_reg=GCH,
                elem_size=4 * C, elem_step=4 * C,
                queue_num=(0 if k % 2 == 0 else 2))
            if g == 0 and k == 0:
                # emitted here (not earlier) so idx casts outrank this DVE
                # work in the scheduler's per-engine stream order
                build_wmath()
            # bilinear: tmp = gt * W4d (pair-packed bcast over ch), corner adds
            gt4 = gt[:].rearrange("p c8 (j ch two) -> p (c8 j) ch two",
                                  j=4, two=2)
            w4b = W4d[:, g * 8:(g + 1) * 8, :, k:k + 1, :].rearrange(
                "p c8 j k two -> p (c8 j) k two").broadcast_to(
                [P, 32, C // 2, 2])
            tmp = tpool.tile([P, 8, 4, C], BF16, tag="btmp")
            tmp_v = tmp[:].rearrange("p c8 j (ch two) -> p (c8 j) ch two",
                                     two=2)
            nc.vector.tensor_tensor(tmp_v, gt4, w4b, Alu.mult)
            t2 = tpool.tile([P, 8, 2, C], BF16, tag="bt2")
            nc.vector.tensor_tensor(t2[:], tmp[:, :, 0:2, :],
                                    tmp[:, :, 2:4, :], Alu.add)
            colk = ckpool.tile([P, 8, C], BF16, tag="colk")
            nc.vector.tensor_tensor(colk[:], t2[:, :, 0, :],
                                    t2[:, :, 1, :], Alu.add)
            for s2 in range(2):
                emit_tr_round(g * 2 + s2, k,
                              colk[:, 4 * s2:4 * (s2 + 1), :])
        if g == 0:
            emit_l2(1)
        finish_gemm(g * 2)
        finish_gemm(g * 2 + 1)

    ctx.close()


# ---------------- harness entry point ----------------

_CACHED_NC = None


def _get_nc():
    global _CACHED_NC
    if _CACHED_NC is None:
        _CACHED_NC = build()
    return _CACHED_NC


def kernel(input, offset, weight):
    """Deformable conv v1 on 8 TRN2 cores, one sample per core.

    input  [8, 256, 64, 64] f32
    offset [8, 18, 64, 64]  f32
    weight [256, 256, 3, 3] f32
    -> [8, 256, 64, 64] f32
    """
    from concourse.bass_utils import run_bass_kernel_spmd
    nc = _get_nc()
    in_maps = host_inputs(input, offset, weight)
    res = run_bass_kernel_spmd(nc, in_maps, core_ids=list(range(8)))
    out = np.stack([res.results[b]["y"].reshape(O, H, W) for b in range(8)])
    return out

